# revision 2
# baseline (speedup 1.0000x reference)
"""MoE router (AutonomousRouter) for TRN2, 8 NeuronCores.

Computes reference:
    act    = einsum('bnd,edc->bnec', x, W)          B,N,D,E,C = 4,2048,2048,8,512
    logits = ||act||_2 over c                       [B,N,E]
    probs  = softmax(logits, -1)
    top-2 routing with capacity 640 (priority = order within k-major (choice, token) sequence)
    out    = stack([dispatch, combine])             [2,B,N,E,640] fp32

Sharding: data-parallel over tokens; core i <- tokens [i*1024, (i+1)*1024) of the
flattened [8192] token axis (= batch b=i//2, half i%2). Weights replicated.

Error budget: the graded metric is absmax/scale with scale ~1 (dispatch ones), so
routing DECISIONS (ordered top-2 per token) must match the fp32 reference exactly,
while combine probabilities only need ~2e-2 absolute. A single fp16 matmul pass
(11-bit mantissa, exact products accumulated in fp32 PSUM) gives logits to ~1.6e-3
absolute — enough to decide all tokens whose top-3 margin exceeds THETA, and to get
probs to ~3e-4. The few hundred tokens with margin < THETA are re-decided on host in
fp64 during the inter-phase glue (the same place the cross-core cumsum offsets are
combined), with near-exact ties (< TIE_TOL, i.e. below fp32 reference resolution)
broken toward the lower expert index. This replaces the previous 3x bf16-split
matmul (hh+hl+lh) with a single fp16 pass -- 3x less PE work.

Phase A (device): fp16 matmul -> square -> reduce = sum-of-squares logits proxy.
Host glue: logits/softmax/top-2 + fp64 refinement of near-tie tokens + exact
integer capacity cumsums + cross-core offsets.
Phase B (device): per-(token,choice) one-hot rows (iota==slot)*{1,prob} built on
DVE and indirect-scattered into the pre-zeroed dense outputs.
"""
import numpy as np

import concourse.bacc as bacc
import concourse.mybir as mybir
from concourse.tile import TileContext
from concourse.bass_utils import run_bass_kernel_spmd

P = 128          # partitions
B, N, D, E, C = 4, 2048, 2048, 8, 512
CAP = 640
NCORES = 8
TOK = (B * N) // NCORES     # tokens per core = 1024
NT = TOK // P               # token tiles per core = 8
KT = D // P                 # contraction tiles = 16

WSCALE = 1024.0   # pre-scale W so its fp16 lo bits stay in normal range
THETA = 0.0128    # flag margin on fp16-pass logits (8x measured max abs err)
TIE_TOL = 1.2e-5  # below fp32-reference resolution: tie -> lower expert index

f32 = mybir.dt.float32

_cache = {}
LAST_IN_MAPS_A = None   # kept for test harness re-runs/profiling
LAST_IN_MAPS_B = None


def _build_phase_a():
    f16 = mybir.dt.float16
    nc = bacc.Bacc("TRN2", target_bir_lowering=False, debug=False, num_devices=NCORES)
    xT = nc.dram_tensor("xT", [D, TOK], f16, kind="ExternalInput")
    w = nc.dram_tensor("w", [E, D, C], f16, kind="ExternalInput")
    ss_out = nc.dram_tensor("ss", [TOK, E], f32, kind="ExternalOutput")

    with TileContext(nc) as tc:
        with (
            tc.tile_pool(name="const", bufs=1) as cpool,
            tc.tile_pool(name="wbuf", bufs=2) as wpool,
            tc.tile_pool(name="work", bufs=3) as spool,
            tc.tile_pool(name="psum", bufs=8, space="PSUM") as psum,
        ):
            # x^T resident in variable k-chunk tiles; W per expert likewise
            # (double-buffered). DMAs are issued in consumption order and the
            # first chunk is a single k-block, so the first matmuls wait on
            # ~0.4MB instead of the full 21MB.
            CHUNKS = [1, 3, 4, 4, 4]           # k-blocks per chunk, sums to KT
            CH0 = [sum(CHUNKS[:i]) for i in range(len(CHUNKS))]  # chunk k-starts
            NCH = len(CHUNKS)

            def _x_chunk(q):
                nk = CHUNKS[q]
                name = f"xq{q}"
                tile_ = cpool.tile([P, nk * TOK], f16, tag=name, name=name)
                nc.sync.dma_start(
                    out=tile_[:].rearrange("p (k n) -> p k n", k=nk),
                    in_=xT.ap()[CH0[q] * P:(CH0[q] + nk) * P, :]
                        .rearrange("(k p) n -> p k n", p=P),
                )
                return tile_

            def _w_chunk(e, q):
                nk = CHUNKS[q]
                tile_ = wpool.tile([P, nk * C], f16, tag=f"wq{q}", name=f"w{e}_{q}")
                nc.sync.dma_start(
                    out=tile_[:].rearrange("p (k c) -> p k c", k=nk),
                    in_=w.ap()[e, CH0[q] * P:(CH0[q] + nk) * P, :]
                        .rearrange("(k p) c -> p k c", p=P),
                )
                return tile_

            # consumption-order issue: W(e0,q0), x(q0), W(e0,q1), x(q1), ...
            w0_q, x_q = [], []
            for q in range(NCH):
                w0_q.append(_w_chunk(0, q))
                x_q.append(_x_chunk(q))

            # per-token-tile sum-of-squares accumulators [128, E]
            ss_tiles = [cpool.tile([P, E], f32, tag=f"ss{t}", name=f"ss{t}")
                        for t in range(NT)]

            # ---- matmul phase: for each expert, 8 token tiles x 16 k-tiles ----
            for e in range(E):
                w_q = w0_q if e == 0 else [_w_chunk(e, q) for q in range(NCH)]
                for t in range(NT):
                    ps = psum.tile([P, C], f32, space="PSUM", tag="ps")
                    for k in range(KT):
                        q = max(i for i in range(NCH) if CH0[i] <= k)
                        kq = k - CH0[q]
                        nc.tensor.matmul(
                            ps[:],
                            lhsT=x_q[q][:, kq * TOK + t * P: kq * TOK + (t + 1) * P],
                            rhs=w_q[q][:, kq * C:(kq + 1) * C],
                            start=(k == 0),
                            stop=(k == KT - 1),
                        )
                    sq = spool.tile([P, C], f32, tag="sq")
                    nc.scalar.activation(sq[:], ps[:], mybir.ActivationFunctionType.Square)
                    red8 = spool.tile([P, 8], f32, tag="red8")
                    nc.vector.tensor_reduce(
                        red8[:], sq[:].rearrange("p (g c) -> p g c", g=8),
                        axis=mybir.AxisListType.X, op=mybir.AluOpType.add,
                    )
                    nc.vector.tensor_reduce(
                        ss_tiles[t][:, e:e + 1], red8[:],
                        axis=mybir.AxisListType.X, op=mybir.AluOpType.add,
                    )

            for t in range(NT):
                nc.sync.dma_start(out=ss_out.ap()[t * P:(t + 1) * P, :],
                                  in_=ss_tiles[t][:])
    nc.compile()
    return nc


def _build_phase_b(cap=CAP):
    """Scatter expansion: dispatch/combine have <=2 nonzero (t,e) rows per
    token; build only those 2048 rows each and indirect-scatter them into the
    pre-zeroed outputs (4x fewer bytes + 4x less DVE than a dense write)."""
    import concourse.bass as bass
    i32 = mybir.dt.int32
    NR = 2 * TOK          # (token x choice) rows per core
    NG = NR // P          # 16 scatter groups of 128 rows
    nc = bacc.Bacc("TRN2", target_bir_lowering=False, debug=False, num_devices=NCORES)
    slot = nc.dram_tensor("slot", [NR, 1], f32, kind="ExternalInput")
    prob = nc.dram_tensor("prob", [NR, 1], f32, kind="ExternalInput")
    ridx = nc.dram_tensor("ridx", [NR, 1], i32, kind="ExternalInput")
    iota_cap = nc.dram_tensor("iota_cap", [P, cap], f32, kind="ExternalInput")
    disp = nc.dram_tensor("disp", [TOK * E, cap], f32, kind="ExternalOutput")
    comb = nc.dram_tensor("comb", [TOK * E, cap], f32, kind="ExternalOutput")

    with TileContext(nc) as tc:
        with (
            tc.tile_pool(name="const", bufs=1) as cpool,
            tc.tile_pool(name="work", bufs=4) as spool,
        ):
            iota_sb = cpool.tile([P, cap], f32, tag="iota")
            nc.sync.dma_start(out=iota_sb[:], in_=iota_cap.ap()[:, :])
            # batched scatter inputs: [NR,1] -> [128, NG] (group-major columns)
            sl = cpool.tile([P, NG], f32, tag="sl")
            nc.sync.dma_start(out=sl[:], in_=slot.ap()[:, 0].rearrange("(g p) -> p g", p=P))
            pr = cpool.tile([P, NG], f32, tag="pr")
            nc.sync.dma_start(out=pr[:], in_=prob.ap()[:, 0].rearrange("(g p) -> p g", p=P))
            ri = cpool.tile([P, NG], i32, tag="ri")
            nc.sync.dma_start(out=ri[:], in_=ridx.ap()[:, 0].rearrange("(g p) -> p g", p=P))
            for g in range(NG):
                drow = spool.tile([P, cap], f32, tag="drow")
                nc.vector.tensor_scalar(drow[:], iota_sb[:], sl[:, g:g + 1], None,
                                        op0=mybir.AluOpType.is_equal)
                crow = spool.tile([P, cap], f32, tag="crow")
                nc.vector.tensor_scalar(crow[:], iota_sb[:], sl[:, g:g + 1], pr[:, g:g + 1],
                                        op0=mybir.AluOpType.is_equal,
                                        op1=mybir.AluOpType.mult)
                nc.gpsimd.indirect_dma_start(
                    out=disp.ap()[:, :],
                    out_offset=bass.IndirectOffsetOnAxis(ap=ri[:, g:g + 1], axis=0),
                    in_=drow[:], in_offset=None)
                nc.gpsimd.indirect_dma_start(
                    out=comb.ap()[:, :],
                    out_offset=bass.IndirectOffsetOnAxis(ap=ri[:, g:g + 1], axis=0),
                    in_=crow[:], in_offset=None)
    nc.compile()
    return nc


def _get(name, builder):
    if name not in _cache:
        _cache[name] = builder()
    return _cache[name]


def kernel(token_inputs, bottleneck_weights, expert_capacity):
    x = np.ascontiguousarray(np.asarray(token_inputs, dtype=np.float32)).reshape(B * N, D)
    w = np.ascontiguousarray(np.asarray(bottleneck_weights, dtype=np.float32))
    cap = int(expert_capacity)
    assert cap > 0

    w16 = (w * WSCALE).astype(np.float16)
    core_ids = list(range(NCORES))
    in_maps_a = []
    for c in core_ids:
        shard_t = np.ascontiguousarray(x[c * TOK:(c + 1) * TOK].T)   # [2048, 1024]
        in_maps_a.append({"xT": shard_t.astype(np.float16), "w": w16})

    global LAST_IN_MAPS_A, LAST_IN_MAPS_B
    LAST_IN_MAPS_A = in_maps_a
    nc_a = _get("a", _build_phase_a)
    res_a = run_bass_kernel_spmd(nc_a, in_maps_a, core_ids)

    # ---- host glue: logits -> decisions (with fp64 refinement of near-ties),
    # exact capacity cumsums, cross-core offsets, phase-B scatter tables.
    ss = np.concatenate([np.asarray(res_a.results[c]["ss"], np.float64)
                         for c in core_ids], axis=0) / (WSCALE * WSCALE)
    l = np.sqrt(ss)                                # [8192, E] fp16-pass logits

    order = np.argsort(-l, axis=1, kind="stable")
    l_srt = np.take_along_axis(l, order, 1)
    margin = np.minimum(l_srt[:, 0] - l_srt[:, 1], l_srt[:, 1] - l_srt[:, 2])
    flagged = np.where(margin < THETA)[0]

    if flagged.size:
        # exact logits for the ambiguous tokens (fp64 ~ exact at this scale)
        xf = x[flagged].astype(np.float64)
        wf = np.ascontiguousarray(w.astype(np.float64).transpose(1, 0, 2)).reshape(D, E * C)
        af = (xf @ wf).reshape(-1, E, C)
        l[flagged] = np.sqrt((af * af).sum(-1))
        # re-rank flagged tokens; near-exact ties (below the fp32 resolution of
        # the reference) go to the lower expert index, matching top_k on probs
        for i in flagged:
            li = l[i]
            o = np.argsort(-li, kind="stable")
            for _ in range(E):
                moved = False
                for j in range(E - 1):
                    if (li[o[j]] - li[o[j + 1]] < TIE_TOL) and o[j] > o[j + 1]:
                        o[j], o[j + 1] = o[j + 1], o[j]
                        moved = True
                if not moved:
                    break
            order[i] = o

    e0 = order[:, 0]
    e1 = order[:, 1]
    ex = np.exp(l - l.max(axis=1, keepdims=True))
    probs = ex / ex.sum(axis=1, keepdims=True)
    arN = np.arange(B * N)
    p0 = probs[arN, e0].astype(np.float32)
    p1 = probs[arN, e1].astype(np.float32)

    # exact integer priorities, replicating the reference's k-major cumsum
    slot0 = np.empty(B * N, np.int64)
    slot1 = np.empty(B * N, np.int64)
    arn = np.arange(N)
    for b in range(B):
        sl_ = slice(b * N, (b + 1) * N)
        idx = np.concatenate([e0[sl_], e1[sl_]])
        oh = (idx[:, None] == np.arange(E)[None, :]).astype(np.int64)
        pri = np.cumsum(oh, axis=0) * oh - 1
        slot0[sl_] = pri[:N][arn, e0[sl_]]
        slot1[sl_] = pri[N:][arn, e1[sl_]]

    ar = np.arange(TOK)
    iota_cap = np.tile(np.arange(cap, dtype=np.float32), (P, 1))
    in_maps_b = []
    for c in core_ids:
        sl_ = slice(c * TOK, (c + 1) * TOK)
        in_maps_b.append({
            "slot": np.concatenate([slot0[sl_], slot1[sl_]]).astype(np.float32)[:, None],
            "prob": np.concatenate([p0[sl_], p1[sl_]]).astype(np.float32)[:, None],
            "ridx": np.concatenate([ar * E + e0[sl_], ar * E + e1[sl_]]).astype(np.int32)[:, None],
            "iota_cap": iota_cap,
        })

    LAST_IN_MAPS_B = in_maps_b
    nc_b = _get(f"b{cap}", lambda: _build_phase_b(cap))
    res_b = run_bass_kernel_spmd(nc_b, in_maps_b, core_ids)

    out = np.empty((2, B, N, E, cap), np.float32)
    for c in core_ids:
        b, h = c // 2, c % 2
        sl_ = slice(h * TOK, (h + 1) * TOK)
        out[0, b, sl_] = res_b.results[c]["disp"].reshape(TOK, E, cap)
        out[1, b, sl_] = res_b.results[c]["comb"].reshape(TOK, E, cap)
    return out
